# revision 38
# baseline (speedup 1.0000x reference)
"""Deductron kernel for Trainium2, 8 NeuronCores, time-sharded.

Math (matching the reference):
    h = sigmoid(W1 @ x + B1); left, right = h[:128], h[128:]
    a_t = left_t * right_t; b_t = 1 - left_t
    u_0 = 0; u_t = a_{t-1} * u_{t-1} + b_{t-1}   (z[:, t] = u_t)
    out = 1 - sigmoid(W2 @ z + B2) = sigmoid(-(W2 @ z + B2))

Sharding: the 65536-frame time axis is split into 8 chunks of 8192 plus a
128-frame left washout halo per core (measured worst-case prod(a) over any
128-step boundary window is e^-182, so the recurrence state forgets its
initial condition well inside the halo; core 0's halo input is zero-padded
and its halo b is scaled by 0 so the state stays exactly 0).

Key implementation points:
  * GEMM1 runs in fp8e4 (e4m3) with MatmulPerfMode.DoubleRow: 256-deep
    contraction per matmul at 0.5 PE cycles/row. W1 is pre-scaled by 8 on
    host (avoids fp8 subnormals); the h-activation applies scale=1/8.
    Host-emulated end-to-end rel err: 2.0e-3 (gate 2e-2).
  * Frames processed in PAIRS of 512-col tiles: one contiguous 512KB input
    DMA per pair (128 descriptors x 4KB) and one 256KB store per pair --
    ~20 DMA instructions total (DMA issue costs ~600ns serial Sync time).
  * PSUM: per-tile [128,2,512] fp32 tiles, two pools x bufs=2 = 8 banks.
    Ring depth 2 decouples the PE from the Activation engine (pair-wide
    PSUM tiles serialize GEMM behind ACT and drop the PE to its lowest
    DVFS p-state -- measured 40% slower matmuls).
  * b = 1-left on GpSimd; a = left*right and the recurrence scan
    (tensor_tensor_scan, fp32 state) on DVE; sigmoids on Scalar.  The
    Activation engine is the steady-state bottleneck at ~4.6us/pair
    (8 sigmoid instrs x ~570ns; 1 elem/cycle/lane at 1.2GHz is a hard
    floor, and no other engine supports activation tables).
  * 8 throwaway matmuls on zeroed scratch warm the PE's DVFS p-state
    during the unavoidable ~11us prologue (framework preamble + first
    input DMA), and the last pair's scan/output chain is split in half
    to shorten the pipeline drain.  Typical: 59-62us (from the 75.1us
    fp16 baseline); rel err 1.9e-3.
"""

import sys

for _p in ("/opt/trn_rl_repo", "/opt/pypackages"):
    if _p not in sys.path:
        sys.path.append(_p)

import numpy as np
import ml_dtypes

# Problem constants (hardcoded per contract).
INPUT_LEN = 512
N_MEM = 128
OUT_LEN = 256
T_TOTAL = 65536
N_CORES = 8
T_LOC = T_TOTAL // N_CORES   # 8192 owned frames per core
HALO = 128                   # washout halo (see module docstring)
TW = 512                     # column tile width (one PSUM bank of fp32)
NPAIR = T_LOC // (2 * TW)    # 8 pairs of owned tiles per core
W_IN = HALO + T_LOC          # 8320
W1_SCALE = 8.0               # host multiplies W1 by this; ACT applies 1/8

F16_NP = np.float16
F8_NP = ml_dtypes.float8_e4m3fn


def _build_nc():
    import concourse.tile as tile
    from concourse import bacc, mybir
    from contextlib import ExitStack

    F32 = mybir.dt.float32
    F16 = mybir.dt.float16
    F8 = mybir.dt.float8e4
    SIG = mybir.ActivationFunctionType.Sigmoid
    MUL = mybir.AluOpType.mult
    ADD = mybir.AluOpType.add
    DR = mybir.MatmulPerfMode.DoubleRow

    nc = bacc.Bacc()
    # DRAM layouts are host-packed so every DMA is fully contiguous.
    x_halo = nc.dram_tensor("x_halo", [128, 4 * HALO], F8, kind="ExternalInput")
    x_main = nc.dram_tensor("x_main", [NPAIR, 128, 4096], F8, kind="ExternalInput")
    w1 = nc.dram_tensor("w1", [128, 1024], F8, kind="ExternalInput")
    w2 = nc.dram_tensor("w2", [128, 256], F16, kind="ExternalInput")
    bias = nc.dram_tensor("bias", [128, 5], F32, kind="ExternalInput")
    out = nc.dram_tensor("out", [NPAIR, 128, 2048], F16, kind="ExternalOutput")

    with ExitStack() as ctx:
        tc = ctx.enter_context(tile.TileContext(nc))
        singles = ctx.enter_context(tc.tile_pool(name="singles", bufs=1))
        xpool = ctx.enter_context(tc.tile_pool(name="xpool", bufs=3))
        hpool = ctx.enter_context(tc.tile_pool(name="hpool", bufs=3))
        opool = ctx.enter_context(tc.tile_pool(name="opool", bufs=2))
        psG = ctx.enter_context(tc.tile_pool(name="psG", bufs=2, space="PSUM"))
        psO = ctx.enter_context(tc.tile_pool(name="psO", bufs=2, space="PSUM"))

        # Persistent recurrence buffers. a_buf/b_buf are written at a +1
        # column offset (a_buf[:, p] = a at input column p-1) so the scan
        # output z[:, p] = u at column p directly.
        a_buf = singles.tile([N_MEM, W_IN + 1], F16)
        b_buf = singles.tile([N_MEM, W_IN + 1], F16)
        z_buf = singles.tile([N_MEM, W_IN], F16)

        # ---- weights / biases (host provides packed layouts); the first
        # two DMAs feed the halo GEMM, so issue them first.
        # w1_sb[p, c, i, h, m] = 8*W1[h*128+m, c*256 + i*128 + p]
        w1_sb = singles.tile([128, 2, 2, 2, 128], F8)
        nc.sync.dma_start(out=w1_sb,
                          in_=w1[:].rearrange("p (c i h m) -> p c i h m",
                                              c=2, i=2, h=2))
        xh = xpool.tile([128, 2, 2, HALO], F8)
        nc.sync.dma_start(out=xh,
                          in_=x_halo[:].rearrange("p (c i w) -> p c i w",
                                                  c=2, i=2))
        # bias cols: [B1a, B1b, -B2a, -B2b, bscale]
        bias_sb = singles.tile([128, 5], F32)
        nc.sync.dma_start(out=bias_sb, in_=bias[:])

        xmr = x_main[:].rearrange("q p (t c i w) -> q p t c i w", t=2, c=2, i=2)
        outr = out[:].rearrange("q p (h w) -> q p h w", h=2)

        # Prefetch pair 0 input before the (later-needed) w2 weights.
        xt0 = xpool.tile([128, 2, 2, 2, TW], F8)
        nc.sync.dma_start(out=xt0, in_=xmr[0])

        # w2_sb[p, h, m] = W2[h*128+m, p]
        w2_sb = singles.tile([128, 2, 128], F16)
        nc.sync.dma_start(out=w2_sb,
                          in_=w2[:].rearrange("p (h m) -> p h m", h=2))

        nc.vector.memset(a_buf[:, 0:1], 0.0)
        nc.vector.memset(b_buf[:, 0:1], 0.0)

        # DVFS warmup: the PE starts in its lowest p-state and only ramps
        # while continuously busy. Real work can't start until the first
        # input DMAs land (~11us: framework preamble + issue + transfer),
        # so run throwaway matmuls on a zeroed scratch tile to have the PE
        # already ramped when the halo GEMM issues.
        scratch = singles.tile([128, 2, TW], F8)
        nc.gpsimd.memset(scratch, 0.0)
        for w in range(4):
            ow = psO.tile([128, 2, TW], F32, name="o")
            for h in range(2):
                nc.tensor.matmul(ow[:, h, :], lhsT=scratch[:, :, 0:128],
                                 rhs=scratch, start=True, stop=True,
                                 perf_mode=DR)

        def phase_c(q):
            # output GEMM + activation + store for pair q (z cols
            # [HALO+1024q, HALO+1024q+1024), out cols [1024q, 1024q+1024))
            zc = HALO + 1024 * q
            ot = opool.tile([128, 2, 1024], F16)
            for t in range(2):
                o = psO.tile([128, 2, TW], F32)
                for h in range(2):
                    nc.tensor.matmul(o[:, h, :], lhsT=w2_sb[:, h, :],
                                     rhs=z_buf[:, zc + TW * t:zc + TW * (t + 1)],
                                     start=True, stop=True)
                nc.scalar.activation(ot[:, 0, TW * t:TW * (t + 1)], o[:, 0, :],
                                     SIG, bias=bias_sb[:, 2:3], scale=-1.0)
                nc.scalar.activation(ot[:, 1, TW * t:TW * (t + 1)], o[:, 1, :],
                                     SIG, bias=bias_sb[:, 3:4], scale=-1.0)
            nc.sync.dma_start(out=outr[q], in_=ot)

        DELAY = 1  # pairs of lead distance between phase A/B and phase C

        # ---- halo tile (columns [0, HALO)) ----
        g = psG.tile([128, 2, TW], F32)
        for c in range(2):
            for h in range(2):
                nc.tensor.matmul(g[:, h, 0:HALO], lhsT=w1_sb[:, c, :, h, :],
                                 rhs=xh[:, c, :, :],
                                 start=(c == 0), stop=(c == 1), perf_mode=DR)
        left = hpool.tile([128, 2, TW], F16)
        right = hpool.tile([128, 2, TW], F16)
        nc.scalar.activation(left[:, 0, 0:HALO], g[:, 0, 0:HALO], SIG,
                             bias=bias_sb[:, 0:1], scale=1.0 / W1_SCALE)
        nc.scalar.activation(right[:, 0, 0:HALO], g[:, 1, 0:HALO], SIG,
                             bias=bias_sb[:, 1:2], scale=1.0 / W1_SCALE)
        nc.gpsimd.tensor_scalar(out=b_buf[:, 1:1 + HALO],
                                in0=left[:, 0, 0:HALO],
                                scalar1=-1.0, scalar2=1.0, op0=MUL, op1=ADD)
        nc.vector.tensor_tensor(out=a_buf[:, 1:1 + HALO],
                                in0=left[:, 0, 0:HALO],
                                in1=right[:, 0, 0:HALO], op=MUL)
        # Halo b *= bscale (0 on core 0 so the state stays exactly 0)
        nc.vector.tensor_scalar(out=b_buf[:, 0:HALO + 1],
                                in0=b_buf[:, 0:HALO + 1],
                                scalar1=bias_sb[:, 4:5], scalar2=None, op0=MUL)
        nc.vector.tensor_tensor_scan(out=z_buf[:, 0:HALO],
                                     data0=a_buf[:, 0:HALO],
                                     data1=b_buf[:, 0:HALO],
                                     initial=0.0, op0=MUL, op1=ADD)

        # ---- owned pairs ----
        for p in range(NPAIR):
            c0 = HALO + 1024 * p
            if p == 0:
                xt = xt0
            else:
                xt = xpool.tile([128, 2, 2, 2, TW], F8)
                nc.sync.dma_start(out=xt, in_=xmr[p])
            left = hpool.tile([128, 2, TW], F16)
            right = hpool.tile([128, 2, TW], F16)
            for t in range(2):
                g = psG.tile([128, 2, TW], F32)
                for c in range(2):
                    for h in range(2):
                        nc.tensor.matmul(
                            g[:, h, :], lhsT=w1_sb[:, c, :, h, :],
                            rhs=xt[:, t, c, :, :],
                            start=(c == 0), stop=(c == 1), perf_mode=DR)
                nc.scalar.activation(left[:, t, :], g[:, 0, :], SIG,
                                     bias=bias_sb[:, 0:1], scale=1.0 / W1_SCALE)
                nc.scalar.activation(right[:, t, :], g[:, 1, :], SIG,
                                     bias=bias_sb[:, 1:2], scale=1.0 / W1_SCALE)
            lf = left[:].rearrange("p a b -> p (a b)")
            rf = right[:].rearrange("p a b -> p (a b)")
            nc.gpsimd.tensor_scalar(out=b_buf[:, c0 + 1:c0 + 1025], in0=lf,
                                    scalar1=-1.0, scalar2=1.0,
                                    op0=MUL, op1=ADD)
            nc.vector.tensor_tensor(out=a_buf[:, c0 + 1:c0 + 1025],
                                    in0=lf, in1=rf, op=MUL)
            if p < NPAIR - 1:
                nc.vector.tensor_tensor_scan(out=z_buf[:, c0:c0 + 1024],
                                             data0=a_buf[:, c0:c0 + 1024],
                                             data1=b_buf[:, c0:c0 + 1024],
                                             initial=z_buf[:, c0 - 1:c0],
                                             op0=MUL, op1=ADD)
                if p - DELAY >= 0:
                    phase_c(p - DELAY)
            else:
                # Final pair: split the scan so the last output GEMM chain
                # can start half a pair earlier (shorter pipeline drain).
                nc.vector.tensor_tensor_scan(out=z_buf[:, c0:c0 + TW],
                                             data0=a_buf[:, c0:c0 + TW],
                                             data1=b_buf[:, c0:c0 + TW],
                                             initial=z_buf[:, c0 - 1:c0],
                                             op0=MUL, op1=ADD)
                phase_c(p - DELAY)
                nc.vector.tensor_tensor_scan(
                    out=z_buf[:, c0 + TW:c0 + 1024],
                    data0=a_buf[:, c0 + TW:c0 + 1024],
                    data1=b_buf[:, c0 + TW:c0 + 1024],
                    initial=z_buf[:, c0 + TW - 1:c0 + TW],
                    op0=MUL, op1=ADD)

        # Drain: last pair's output, store per half-pair to shorten the tail.
        q = NPAIR - 1
        zc = HALO + 1024 * q
        outh = out[:].rearrange("q p (h t w) -> q p h t w", h=2, t=2)
        for t in range(2):
            o = psO.tile([128, 2, TW], F32, name="o")
            for h in range(2):
                nc.tensor.matmul(o[:, h, :], lhsT=w2_sb[:, h, :],
                                 rhs=z_buf[:, zc + TW * t:zc + TW * (t + 1)],
                                 start=True, stop=True)
            otl = opool.tile([128, 2, TW], F16)
            nc.scalar.activation(otl[:, 0, :], o[:, 0, :], SIG,
                                 bias=bias_sb[:, 2:3], scale=-1.0)
            nc.scalar.activation(otl[:, 1, :], o[:, 1, :], SIG,
                                 bias=bias_sb[:, 3:4], scale=-1.0)
            nc.sync.dma_start(out=outh[q, :, :, t, :], in_=otl)

    nc.finalize()
    return nc


def _make_in_maps(inputs, W1, B1, W2, B2):
    inputs = np.asarray(inputs, dtype=np.float32)
    W1 = np.asarray(W1, dtype=np.float32)
    B1 = np.asarray(B1, dtype=np.float32)
    W2 = np.asarray(W2, dtype=np.float32)
    B2 = np.asarray(B2, dtype=np.float32)

    x8 = inputs.astype(F8_NP)
    # w1[p, c, i, h, m] = 8*W1[h*128+m, c*256+i*128+p]
    w1p = np.ascontiguousarray(
        (W1 * W1_SCALE).astype(F8_NP)
        .reshape(2, 128, 2, 2, 128)            # h, m, c, i, p
        .transpose(4, 2, 3, 0, 1)              # p, c, i, h, m
        .reshape(128, 1024))
    # w2[p, h, m] = W2[h*128+m, p]
    w2p = np.ascontiguousarray(
        W2.astype(F16_NP).reshape(2, 128, 128)  # h, m, p
        .transpose(2, 0, 1).reshape(128, 256))
    biasc = np.empty((128, 5), np.float32)
    biasc[:, 0] = B1[:128, 0]
    biasc[:, 1] = B1[128:, 0]
    biasc[:, 2] = -B2[:128, 0]
    biasc[:, 3] = -B2[128:, 0]

    in_maps = []
    for i in range(N_CORES):
        s = i * T_LOC
        lo = s - HALO
        if lo < 0:
            xs = np.concatenate(
                [np.zeros((INPUT_LEN, -lo), F8_NP), x8[:, :s + T_LOC]], axis=1)
        else:
            xs = x8[:, lo:s + T_LOC]
        xr = xs.reshape(2, 2, 128, W_IN)                  # c, i, p, col
        xhm = np.ascontiguousarray(
            xr[:, :, :, :HALO].transpose(2, 0, 1, 3).reshape(128, 4 * HALO))
        xm = np.ascontiguousarray(
            xr[:, :, :, HALO:].reshape(2, 2, 128, NPAIR, 2, TW)
            .transpose(3, 2, 4, 0, 1, 5)                  # pair, p, t, c, i, w
            .reshape(NPAIR, 128, 4096))
        b = biasc.copy()
        b[:, 4] = 0.0 if i == 0 else 1.0
        in_maps.append({
            "x_halo": xhm, "x_main": xm,
            "w1": w1p, "w2": w2p, "bias": b,
        })
    return in_maps


def _run(inputs, W1, B1, W2, B2, trace=False, **kw):
    from concourse.bass_utils import run_bass_kernel_spmd

    nc = _build_nc()
    in_maps = _make_in_maps(inputs, W1, B1, W2, B2)
    res = run_bass_kernel_spmd(nc, in_maps, list(range(N_CORES)), trace=trace, **kw)
    parts = []
    for r in res.results:
        o = np.asarray(r["out"]).astype(np.float32)       # (NPAIR, 128, 2048)
        o = (o.reshape(NPAIR, 128, 2, 2, TW)              # pair, p, h, t, w
             .transpose(2, 1, 0, 3, 4)                    # h, p, pair, t, w
             .reshape(OUT_LEN, T_LOC))
        parts.append(o)
    full = np.concatenate(parts, axis=1)
    return full, res


def kernel(inputs, W1, B1, W2, B2):
    full, _ = _run(inputs, W1, B1, W2, B2, trace=False)
    return full.astype(np.float32, copy=False)
